# revision 7
# baseline (speedup 1.0000x reference)
"""GCN-5 message-passing kernel for Trainium2, 8-core SPMD Bass/Tile.

Strategy (graph-parallel, per the sharding hint):
  - batch is sorted, so graphs are contiguous node ranges.  Core c owns graphs
    [125c, 125(c+1)) and their nodes; edges are assigned to the core owning the
    dst node.  Pooling / layernorm / output head are fully local per core.
  - Per layer each core computes z = h @ W for its own node slice on the
    TensorEngine, the slices are AllGathered (device collective), and each core
    gathers z[src] rows for its edges with bulk dma_gather (SWDGE): one call
    per (source 2-core region, chunk of TC dst tiles), thousands of rows per
    instruction instead of one 128-row indirect DMA per edge block.  Source
    regions span 2 cores (2S = 25600 rows) so row indices fit dma_gather's
    int16 index format.
  - Scatter-add per 128-node dst tile is a one-hot matmul on the TensorEngine
    with PSUM accumulation.  The gathered messages and one-hot masks are fp16
    (2x PE/DVE rate; one-hot values and dst positions are exact in fp16).
    Edges are pre-sorted by (dst tile, src region) and padded to 128-slot
    blocks on the host; padded slots gather region row 0 with norm=0.
  - Block structure (blocks per tile/region) is the max over cores so the
    single SPMD program fits all 8 cores; per-core padding lives in the data.
  - The symmetric normalization deg^-1/2 and all index inputs are precomputed
    on the host from edge_index/batch.
"""
import sys
import types
import contextlib

import numpy as np

sys.path.insert(0, "/opt/trn_rl_repo")

import concourse.bass as bass
import concourse.tile as tile
from concourse import mybir, library_config
from concourse.masks import make_identity
from concourse.vector_clock import ScopedClock

F32 = mybir.dt.float32
F16 = mybir.dt.float16
I16 = mybir.dt.int16
M = 8  # NeuronCores
H = 64
NREG = 4   # source regions of 2 cores each (2S rows <= int16 max)
TC = 7     # dst tiles per gather chunk

# ---------------------------------------------------------------------------
# Environment fixes for this container
# ---------------------------------------------------------------------------

def _install_env_fixes():
    import concourse.tile as tile_mod

    def _patched_drain_and_barrier(self, tick_clock, wait_clock):
        # this walrus build allows a single sync-wait per TPB_CTRL Drain;
        # split the Tile tail-drain's waits across multiple drains.
        nc = self.nc
        drain_inst = nc.sync.drain()
        wait_clock.add_sem_waits(drain_inst.ins,
                                 ScopedClock({None: tick_clock.global_clock}))
        si = drain_inst.ins.sync_info
        waits = list(si.on_wait or [])
        if len(waits) > 1:
            si.on_wait[:] = waits[:1]
            for w in waits[1:]:
                d2 = nc.sync.drain()
                if d2.ins.sync_info is None:
                    d2.ins.sync_info = mybir.SyncInfo(on_wait=[w], on_update=[])
                else:
                    d2.ins.sync_info.on_wait.append(w)
        nc.all_engine_barrier()
        assert self.sems is not None
        popped = nc._tile_sem_poison_stack.pop()
        assert popped is self._sem_poison
        nc.clear_and_free_semaphores(list(self.sems.allocated().values()))
        nc.all_engine_barrier()

    tile_mod.TileContext._drain_and_barrier = _patched_drain_and_barrier

    _orig_lower_ordered = tile_mod.TileContext._lower_ordered_insts

    def _split_multiwait_lower(self, ordered):
        nc = self.nc
        for bbname, insts in ordered.items():
            newlist = []
            changed = False
            for inst in insts:
                si = getattr(inst, "sync_info", None)
                eng = getattr(inst, "engine", None)
                if (si is not None and si.on_wait and len(si.on_wait) > 1
                        and eng is not None and eng != mybir.EngineType.Unassigned
                        and inst.is_executable()):
                    waits = list(si.on_wait)
                    si.on_wait[:] = waits[-1:]
                    for w in waits[:-1]:
                        nop = mybir.InstNoOp(
                            name=nc.get_next_instruction_name(), engine=eng)
                        nop.sync_info = mybir.SyncInfo(on_wait=[w], on_update=[])
                        try:
                            nc.register_instruction(nop, overwrite=True)
                        except Exception:
                            pass
                        newlist.append(nop)
                    changed = True
                newlist.append(inst)
            if changed:
                insts[:] = newlist
        return _orig_lower_ordered(self, ordered)

    if getattr(tile_mod.TileContext._lower_ordered_insts, "__name__", "") != \
            "_split_multiwait_lower":
        tile_mod.TileContext._lower_ordered_insts = _split_multiwait_lower

    # NTFF profile hook (lets trace=True work under axon); best-effort.
    if "antenv.axon_hooks" not in sys.modules:
        try:
            from trn_agent_boot.trn_boot import _ntff_profile_via_ctypes
            hook = _ntff_profile_via_ctypes("/opt/axon/libaxon_pjrt.so")
            mod = types.ModuleType("antenv.axon_hooks")
            mod.get_axon_ntff_profile_hook = lambda: hook
            mod.set_axon_ntff_profile_hook = lambda h: None
            sys.modules["antenv.axon_hooks"] = mod
            import antenv
            antenv.axon_hooks = mod
        except Exception:
            pass


_install_env_fixes()

# ---------------------------------------------------------------------------
# Host preprocessing
# ---------------------------------------------------------------------------

def preprocess(x, edge_index, batch, n_graphs):
    N = x.shape[0]
    GPC = n_graphs // M
    src = np.concatenate([edge_index[0], np.arange(N, dtype=np.int64)])
    dst = np.concatenate([edge_index[1], np.arange(N, dtype=np.int64)])
    deg = np.bincount(dst, minlength=N).astype(np.float32)
    dis = 1.0 / np.sqrt(deg)
    norm = (dis[src] * dis[dst]).astype(np.float32)

    batch = np.asarray(batch)
    owner = (batch // GPC).astype(np.int64)
    node_start = np.searchsorted(batch, np.arange(M) * GPC)
    node_end = np.searchsorted(batch, np.arange(M) * GPC + GPC)
    n_c = node_end - node_start
    T = int(np.ceil(n_c.max() / 128))
    S = T * 128
    R = 2 * S  # rows per source region
    assert R <= 32767, R
    local_pos = np.arange(N) - node_start[owner]
    padded_idx = (owner * S + local_pos).astype(np.int64)

    eo = owner[dst]
    ld_all = local_pos[dst]
    grow = padded_idx[src]          # global row in zfull
    rg_all = grow // R              # source region
    rloc_all = grow % R             # region-local row (int16 range)

    # uniform blocks-per-(tile, region): max over cores
    cnt = np.zeros((M, T, NREG), np.int64)
    for c in range(M):
        sel = eo == c
        np.add.at(cnt[c], (ld_all[sel] // 128, rg_all[sel]), 1)
    b_u = -(-cnt.max(axis=0) // 128)          # [T, NREG] blocks
    assert (b_u.sum(axis=1) > 0).all()
    boff = np.zeros((T + 1, NREG), np.int64)  # region-block offsets by tile
    boff[1:] = np.cumsum(b_u, axis=0)
    NB_r = boff[T]                            # blocks per region
    RB = np.concatenate([[0], np.cumsum(NB_r)])  # region col offsets
    NBtot = int(RB[-1])

    G = -(-T // TC)
    # chunk block ranges per region: CB[g][r] = boff[g*TC][r]
    CB = np.stack([boff[min(g * TC, T)] for g in range(G + 1)])

    cnt_g = np.bincount(batch, minlength=n_graphs).astype(np.float32)

    in_maps = []
    for c in range(M):
        sel = np.nonzero(eo == c)[0]
        en, ld = norm[sel], ld_all[sel]
        rg, rloc = rg_all[sel], rloc_all[sel]
        tid = ld // 128
        order = np.lexsort((rg, tid))
        en, ld, rg, rloc, tid = en[order], ld[order], rg[order], rloc[order], tid[order]

        ccnt = cnt[c]                          # [T, NREG] this core's counts
        flat_cnt = ccnt.reshape(-1)
        starts = np.concatenate([[0], np.cumsum(flat_cnt)])[:-1]
        pos = np.arange(len(en)) - np.repeat(starts, flat_cnt)
        slot = (RB[rg] + boff[tid, rg]) * 128 + \
            (np.zeros(len(en), np.int64)) + pos
        # pos counts within the (tid, rg) group in sorted order

        idx_flat = np.zeros(NBtot * 128, np.int16)
        nrm_flat = np.zeros(NBtot * 128, np.float32)
        ldst_flat = np.zeros(NBtot * 128, np.float16)
        idx_flat[slot] = rloc.astype(np.int16)
        nrm_flat[slot] = en
        ldst_flat[slot] = (ld % 128).astype(np.float16)

        # idx packed for dma_gather: element i -> [i % 16, i // 16], x8 copies
        C = NBtot * 8
        pat = idx_flat.reshape(C, 16).T
        idx_pk = np.zeros((128, C), np.int16)
        for k in range(8):
            idx_pk[k * 16:(k + 1) * 16] = pat

        def to_cols(a):
            return np.ascontiguousarray(
                a.reshape(NBtot, 128).T)           # [128, NBtot]

        xT = np.zeros((128, S), np.float32)
        xs = x[node_start[c]:node_end[c]]
        xT[:, :n_c[c]] = xs.T

        gflat = np.full(T * 128, -1.0, np.float32)
        gflat[:n_c[c]] = batch[node_start[c]:node_end[c]] - c * GPC
        gcol = gflat.reshape(T, 128).T

        invcnt = np.zeros((128, 1), np.float32)
        invcnt[:GPC, 0] = 1.0 / np.maximum(cnt_g[c * GPC:(c + 1) * GPC], 1.0)

        iota = np.broadcast_to(np.arange(128, dtype=np.float32), (128, 128)).copy()
        iota16 = iota.astype(np.float16)

        in_maps.append(dict(
            xT=xT, idx=idx_pk, nrm=to_cols(nrm_flat), ldst=to_cols(ldst_flat),
            gcol=np.ascontiguousarray(gcol), invcnt=invcnt, iota=iota,
            iota16=iota16,
        ))
    meta = dict(T=T, S=S, GPC=GPC, b_u=b_u, boff=boff, RB=RB, NBtot=NBtot,
                G=G, CB=CB)
    return in_maps, meta


def make_weight_inputs(W1, b1, Wh, bh, Wout, bout):
    return dict(
        W1=np.ascontiguousarray(W1, np.float32),
        Wh=np.ascontiguousarray(Wh, np.float32),
        b14=np.ascontiguousarray(
            np.stack([b1, bh[0], bh[1], bh[2]], axis=1), np.float32),
        b5rep=np.broadcast_to(np.asarray(bh[3], np.float32), (128, H)).copy(),
        woutrep=np.broadcast_to(np.asarray(Wout, np.float32)[:, 0], (128, H)).copy(),
    ), dict(bout=np.asarray(bout, np.float32))


# ---------------------------------------------------------------------------
# Bass program
# ---------------------------------------------------------------------------

def build_nc(meta, weights):
    T, S = meta["T"], meta["S"]
    b_u, boff, RB, NBtot = meta["b_u"], meta["boff"], meta["RB"], meta["NBtot"]
    G, CB = meta["G"], meta["CB"]
    R = 2 * S
    BMAXT = int(b_u.sum(axis=1).max())            # max blocks per tile
    NBGR = int((CB[1:] - CB[:-1]).max())          # max blocks per (chunk, region)

    nc = bass.Bass("TRN2", target_bir_lowering=False, num_swdge_queues=4)

    xT_d = nc.declare_dram_parameter("xT", [128, S], F32, isOutput=False)
    idx_d = nc.declare_dram_parameter("idx", [128, NBtot * 8], I16, isOutput=False)
    nrm_d = nc.declare_dram_parameter("nrm", [128, NBtot], F32, isOutput=False)
    ldst_d = nc.declare_dram_parameter("ldst", [128, NBtot], F16, isOutput=False)
    gcol_d = nc.declare_dram_parameter("gcol", [128, T], F32, isOutput=False)
    invcnt_d = nc.declare_dram_parameter("invcnt", [128, 1], F32, isOutput=False)
    iota_d = nc.declare_dram_parameter("iota", [128, 128], F32, isOutput=False)
    iota16_d = nc.declare_dram_parameter("iota16", [128, 128], F16, isOutput=False)
    W1_d = nc.declare_dram_parameter("W1", [128, H], F32, isOutput=False)
    Wh_d = nc.declare_dram_parameter("Wh", [4, H, H], F32, isOutput=False)
    b14_d = nc.declare_dram_parameter("b14", [H, 4], F32, isOutput=False)
    b5rep_d = nc.declare_dram_parameter("b5rep", [128, H], F32, isOutput=False)
    woutrep_d = nc.declare_dram_parameter("woutrep", [128, H], F32, isOutput=False)
    out_d = nc.declare_dram_parameter("out", [128, 1], F32, isOutput=True)
    bout = float(weights["bout"][0])

    with tile.TileContext(nc) as tc:
        with contextlib.ExitStack() as ctx:
            zfull_pool = ctx.enter_context(tc.tile_pool(name="zfull", bufs=1, space="DRAM"))
            dram = ctx.enter_context(tc.tile_pool(name="dram", bufs=1, space="DRAM"))
            const = ctx.enter_context(tc.tile_pool(name="const", bufs=1))
            xp = ctx.enter_context(tc.tile_pool(name="xp", bufs=3))
            gp = ctx.enter_context(tc.tile_pool(name="gp", bufs=2))
            g16p = ctx.enter_context(tc.tile_pool(name="g16p", bufs=2))
            mp = ctx.enter_context(tc.tile_pool(name="mp", bufs=3))
            zp = ctx.enter_context(tc.tile_pool(name="zp", bufs=3))
            hp = ctx.enter_context(tc.tile_pool(name="hp", bufs=3))
            ep = ctx.enter_context(tc.tile_pool(name="ep", bufs=2))
            ps_agg = ctx.enter_context(tc.tile_pool(name="ps_agg", bufs=2, space="PSUM"))
            ps_z = ctx.enter_context(tc.tile_pool(name="ps_z", bufs=2, space="PSUM"))
            ps_misc = ctx.enter_context(tc.tile_pool(name="ps_misc", bufs=1, space="PSUM"))

            zfulls = [zfull_pool.tile([M * S, H], F32, addr_space="Shared",
                                      name=f"zfull{k}", tag=f"zfull{k}")
                      for k in range(5)]
            bounces = [dram.tile([S, H], F32, name=f"bounce{k}", tag=f"bounce{k}")
                       for k in range(5)]

            idx_sb = const.tile([128, NBtot * 8], I16)
            nc.sync.dma_start(idx_sb[:], idx_d[:])
            nrm_sb = const.tile([128, NBtot], F32)
            nc.sync.dma_start(nrm_sb[:], nrm_d[:])
            ldst_sb = const.tile([128, NBtot], F16)
            nc.sync.dma_start(ldst_sb[:], ldst_d[:])
            gcol_sb = const.tile([128, T], F32)
            nc.sync.dma_start(gcol_sb[:], gcol_d[:])
            invcnt_sb = const.tile([128, 1], F32)
            nc.sync.dma_start(invcnt_sb[:], invcnt_d[:])
            iota_sb = const.tile([128, 128], F32)
            nc.sync.dma_start(iota_sb[:], iota_d[:])
            iota16_sb = const.tile([128, 128], F16)
            nc.sync.dma_start(iota16_sb[:], iota16_d[:])
            W1_sb = const.tile([128, H], F32)
            nc.sync.dma_start(W1_sb[:], W1_d[:])
            Wh2_sb = const.tile([H, 4 * H], F32)
            for k in range(4):
                nc.sync.dma_start(Wh2_sb[:, k * H:(k + 1) * H], Wh_d[k, :, :])
            b14_sb = const.tile([H, 4], F32)
            nc.sync.dma_start(b14_sb[:], b14_d[:])
            b5rep_sb = const.tile([128, H], F32)
            nc.sync.dma_start(b5rep_sb[:], b5rep_d[:])
            woutrep_sb = const.tile([128, H], F32)
            nc.sync.dma_start(woutrep_sb[:], woutrep_d[:])
            ident = const.tile([H, H], F32)
            make_identity(nc, ident[:])
            nc.gpsimd.load_library(library_config.mlp)
            nvals = sorted({int(CB[g + 1][r] - CB[g][r]) * 128
                            for g in range(G) for r in range(NREG)
                            if CB[g + 1][r] > CB[g][r]})
            nreg = {v: nc.gpsimd.to_reg(v) for v in nvals}

            # layer 0: z0 = x @ W1 for the local slice, then AllGather
            for t in range(T):
                xt = xp.tile([128, 128], F32, tag="xt")
                nc.sync.dma_start(xt[:], xT_d[:, t * 128:(t + 1) * 128])
                pz = ps_z.tile([128, H], F32, space="PSUM", tag="pz")
                nc.tensor.matmul(out=pz[:], lhsT=xt[:], rhs=W1_sb[:], start=True, stop=True)
                zt = zp.tile([128, H], F32, tag="zt")
                nc.scalar.copy(zt[:], pz[:])
                nc.sync.dma_start(bounces[0][t * 128:(t + 1) * 128, :], zt[:])
            nc.gpsimd.collective_compute(
                "AllGather", mybir.AluOpType.bypass,
                replica_groups=[list(range(M))],
                ins=[bounces[0][:]], outs=[zfulls[0][:]])

            # layers 1..5: bulk-gather z[src] per (chunk, region), fp16
            # messages, one-hot scatter matmul per dst tile
            for layer in range(1, 6):
                pool_acc = None
                if layer == 5:
                    pool_acc = const.tile([H, 128], F32, name="pool_acc")
                    nc.vector.memset(pool_acc[:], 0.0)
                    h5all = const.tile([128, T * H], F32, name="h5all")
                for g in range(G):
                    t0, t1 = g * TC, min((g + 1) * TC, T)
                    gbufs, g16s = [], []
                    for r in range(NREG):
                        nb = int(CB[g + 1][r] - CB[g][r])
                        if nb == 0:
                            gbufs.append(None)
                            g16s.append(None)
                            continue
                        gb = gp.tile([128, NBGR * H], F32, tag=f"g{r}")
                        c0 = int(RB[r] + CB[g][r])
                        nc.gpsimd.dma_gather(
                            out_ap=gb[:, :nb * H].rearrange("p (b e) -> p b e", e=H),
                            in_ap=zfulls[layer - 1][r * R:(r + 1) * R, :],
                            idxs_ap=idx_sb[:, c0 * 8:(c0 + nb) * 8],
                            num_idxs=nb * 128,
                            num_idxs_reg=nreg[nb * 128],
                            elem_size=H,
                            single_packet=False,
                            queue_num=1 + (g * NREG + r) % 3,
                        )
                        g16 = g16p.tile([128, NBGR * H], F16, tag=f"h{r}")
                        nrm3 = nrm_sb[:, c0:c0 + nb, None].to_broadcast([128, nb, H])
                        nc.vector.tensor_tensor(
                            out=g16[:, :nb * H].rearrange("p (b e) -> p b e", e=H),
                            in0=gb[:, :nb * H].rearrange("p (b e) -> p b e", e=H),
                            in1=nrm3, op=mybir.AluOpType.mult)
                        gbufs.append(gb)
                        g16s.append(g16)

                    for t in range(t0, t1):
                        sumb = int(b_u[t].sum())
                        m01 = mp.tile([128, BMAXT * 128], F16, tag="m01")
                        jb = 0
                        for r in range(NREG):
                            b = int(b_u[t][r])
                            if b == 0:
                                continue
                            cc = int(RB[r] + boff[t][r])
                            ldst3 = ldst_sb[:, cc:cc + b, None].to_broadcast([128, b, 128])
                            iota3 = iota16_sb[:, None, :].to_broadcast([128, b, 128])
                            m3 = m01[:, jb * 128:(jb + b) * 128].rearrange(
                                "p (b q) -> p b q", b=b)
                            nc.vector.tensor_tensor(out=m3, in0=iota3, in1=ldst3,
                                                    op=mybir.AluOpType.is_equal)
                            jb += b
                        if layer < 5:
                            pT = ps_agg.tile([H, 128], F32, space="PSUM", tag="pT")
                            jb = 0
                            for r in range(NREG):
                                b = int(b_u[t][r])
                                if b == 0:
                                    continue
                                lo = int(boff[t][r] - CB[g][r])
                                for j in range(b):
                                    nc.tensor.matmul(
                                        out=pT[:],
                                        lhsT=g16s[r][:, (lo + j) * H:(lo + j + 1) * H],
                                        rhs=m01[:, jb * 128:(jb + 1) * 128],
                                        start=(jb == 0), stop=(jb == sumb - 1))
                                    jb += 1
                            hT = hp.tile([H, 128], F32, tag="hT")
                            nc.scalar.activation(hT[:], pT[:],
                                                 mybir.ActivationFunctionType.Relu,
                                                 bias=b14_sb[:, layer - 1:layer])
                            pz = ps_z.tile([128, H], F32, space="PSUM", tag="pz")
                            nc.tensor.matmul(out=pz[:], lhsT=hT[:],
                                             rhs=Wh2_sb[:, (layer - 1) * H:layer * H],
                                             start=True, stop=True)
                            zt = zp.tile([128, H], F32, tag="zt2")
                            nc.scalar.copy(zt[:], pz[:])
                            nc.sync.dma_start(bounces[layer][t * 128:(t + 1) * 128, :], zt[:])
                        else:
                            p5 = ps_agg.tile([128, H], F32, space="PSUM", tag="pT")
                            jb = 0
                            for r in range(NREG):
                                b = int(b_u[t][r])
                                if b == 0:
                                    continue
                                lo = int(boff[t][r] - CB[g][r])
                                for j in range(b):
                                    nc.tensor.matmul(
                                        out=p5[:],
                                        lhsT=m01[:, jb * 128:(jb + 1) * 128],
                                        rhs=g16s[r][:, (lo + j) * H:(lo + j + 1) * H],
                                        start=(jb == 0), stop=(jb == sumb - 1))
                                    jb += 1
                            h5 = h5all[:, t * H:(t + 1) * H]
                            nc.vector.tensor_tensor(out=h5, in0=p5[:], in1=b5rep_sb[:],
                                                    op=mybir.AluOpType.add)
                            nc.scalar.activation(h5, h5, mybir.ActivationFunctionType.Relu)
                if layer < 5:
                    nc.gpsimd.collective_compute(
                        "AllGather", mybir.AluOpType.bypass,
                        replica_groups=[list(range(M))],
                        ins=[bounces[layer][:]], outs=[zfulls[layer][:]])

            # mean-pool per graph (one-hot matmul per tile, accumulate in SBUF)
            for t in range(T):
                pt = mp.tile([128, 128], F32, tag="pt")
                nc.vector.tensor_tensor(
                    out=pt[:], in0=iota_sb[:],
                    in1=gcol_sb[:, t:t + 1].to_broadcast([128, 128]),
                    op=mybir.AluOpType.is_equal)
                ppool = ps_misc.tile([H, 128], F32, space="PSUM", tag="pool")
                nc.tensor.matmul(out=ppool[:], lhsT=h5all[:, t * H:(t + 1) * H],
                                 rhs=pt[:], start=True, stop=True)
                nc.vector.tensor_tensor(out=pool_acc[:], in0=pool_acc[:],
                                        in1=ppool[:], op=mybir.AluOpType.add)

            # transpose, scale by 1/cnt, layernorm, output head
            ptr = ps_z.tile([128, H], F32, space="PSUM", tag="pz")
            nc.tensor.transpose(out=ptr[:], in_=pool_acc[:], identity=ident[:])
            pooled = ep.tile([128, H], F32, tag="pooled")
            nc.vector.tensor_scalar(out=pooled[:], in0=ptr[:], scalar1=invcnt_sb[:, 0:1],
                                    scalar2=None, op0=mybir.AluOpType.mult)
            mu = ep.tile([128, 1], F32, tag="mu")
            nc.vector.tensor_reduce(out=mu[:], in_=pooled[:], axis=mybir.AxisListType.X,
                                    op=mybir.AluOpType.add)
            nc.vector.tensor_scalar(out=mu[:], in0=mu[:], scalar1=1.0 / H, scalar2=None,
                                    op0=mybir.AluOpType.mult)
            xc = ep.tile([128, H], F32, tag="xc")
            nc.vector.tensor_scalar(out=xc[:], in0=pooled[:], scalar1=mu[:, 0:1],
                                    scalar2=None, op0=mybir.AluOpType.subtract)
            sq = ep.tile([128, H], F32, tag="sq")
            nc.scalar.activation(sq[:], xc[:], mybir.ActivationFunctionType.Square)
            var = ep.tile([128, 1], F32, tag="var")
            nc.vector.tensor_reduce(out=var[:], in_=sq[:], axis=mybir.AxisListType.X,
                                    op=mybir.AluOpType.add)
            nc.vector.tensor_scalar(out=var[:], in0=var[:], scalar1=1.0 / H, scalar2=None,
                                    op0=mybir.AluOpType.mult)
            eps_col = ep.tile([128, 1], F32, tag="eps")
            nc.gpsimd.memset(eps_col[:], 1e-5)
            std = ep.tile([128, 1], F32, tag="std")
            nc.scalar.activation(std[:], var[:], mybir.ActivationFunctionType.Sqrt,
                                 bias=eps_col[:, 0:1])
            rstd = ep.tile([128, 1], F32, tag="rstd")
            nc.vector.reciprocal(rstd[:], std[:])
            ln = ep.tile([128, H], F32, tag="ln")
            nc.vector.tensor_scalar(out=ln[:], in0=xc[:], scalar1=rstd[:, 0:1],
                                    scalar2=None, op0=mybir.AluOpType.mult)
            y = ep.tile([128, H], F32, tag="y")
            nc.vector.tensor_tensor(out=y[:], in0=ln[:], in1=woutrep_sb[:],
                                    op=mybir.AluOpType.mult)
            yr = ep.tile([128, 1], F32, tag="yr")
            nc.vector.tensor_reduce(out=yr[:], in_=y[:], axis=mybir.AxisListType.X,
                                    op=mybir.AluOpType.add)
            nc.vector.tensor_scalar(out=yr[:], in0=yr[:], scalar1=bout, scalar2=None,
                                    op0=mybir.AluOpType.add)
            nc.sync.dma_start(out_d[:], yr[:])
    mybir.codegen_inst_isa_subclasses(nc)
    return nc


# ---------------------------------------------------------------------------
# Entry point
# ---------------------------------------------------------------------------

def kernel(x, edge_index, batch, W1, b1, Wh, bh, Wout, bout):
    from concourse.bass_utils import run_bass_kernel_spmd

    x = np.asarray(x, np.float32)
    edge_index = np.asarray(edge_index)
    batch = np.asarray(batch)
    n_graphs = 1000

    in_maps, meta = preprocess(x, edge_index, batch, n_graphs)
    wmaps, wmeta = make_weight_inputs(W1, b1, Wh, bh, Wout, bout)
    nc = build_nc(meta, dict(bout=wmeta["bout"]))
    for im in in_maps:
        im.update(wmaps)

    import time
    last_err = None
    for attempt in range(3):
        try:
            res = run_bass_kernel_spmd(nc, in_maps, core_ids=list(range(M)))
            break
        except Exception as e:  # transient terminal hiccups / device recovery
            last_err = e
            time.sleep(30 * (attempt + 1))
    else:
        raise last_err

    GPC = meta["GPC"]
    out = np.concatenate([res.results[c]["out"][:GPC] for c in range(M)], axis=0)
    return np.ascontiguousarray(out, np.float32)


# revision 14
# speedup vs baseline: 1.1468x; 1.1468x over previous
"""GCN-5 message-passing kernel for Trainium2, 8-core SPMD Bass/Tile.

Strategy (graph-parallel, per the sharding hint):
  - batch is sorted, so graphs are contiguous node ranges.  Core c owns graphs
    [125c, 125(c+1)) and their nodes; edges are assigned to the core owning the
    dst node.  Pooling / layernorm / output head are fully local per core.
  - Per layer each core computes z = h @ W for its own node slice on the
    TensorEngine, the slices are AllGathered (device collective), and each core
    gathers z[src] rows for its edges with bulk dma_gather (SWDGE): one call
    per (source 2-core region, chunk of TC dst tiles), thousands of rows per
    instruction instead of one 128-row indirect DMA per edge block.  Source
    regions span 2 cores (2S = 25600 rows) so row indices fit dma_gather's
    int16 index format.
  - Scatter-add per 128-node dst tile is a one-hot matmul on the TensorEngine
    with PSUM accumulation.  The gathered messages and one-hot masks are fp16
    (2x PE/DVE rate; one-hot values and dst positions are exact in fp16).
    Edges are pre-sorted by (dst tile, src region) and padded to 128-slot
    blocks on the host; padded slots gather region row 0 with norm=0.
  - Block structure (blocks per tile/region) is the max over cores so the
    single SPMD program fits all 8 cores; per-core padding lives in the data.
  - The symmetric normalization deg^-1/2 and all index inputs are precomputed
    on the host from edge_index/batch.
"""
import sys
import types
import contextlib

import numpy as np

sys.path.insert(0, "/opt/trn_rl_repo")

import concourse.bass as bass
import concourse.tile as tile
from concourse import mybir, library_config
from concourse.masks import make_identity
from concourse.vector_clock import ScopedClock

F32 = mybir.dt.float32
F16 = mybir.dt.float16
I16 = mybir.dt.int16
M = 8  # NeuronCores
H = 64
NREG = 4   # source regions of 2 cores each (2S rows <= int16 max)
TC = 7     # dst tiles per gather chunk

# ---------------------------------------------------------------------------
# Environment fixes for this container
# ---------------------------------------------------------------------------

def _install_env_fixes():
    import concourse.tile as tile_mod

    def _patched_drain_and_barrier(self, tick_clock, wait_clock):
        # this walrus build allows a single sync-wait per TPB_CTRL Drain;
        # split the Tile tail-drain's waits across multiple drains.
        nc = self.nc
        drain_inst = nc.sync.drain()
        wait_clock.add_sem_waits(drain_inst.ins,
                                 ScopedClock({None: tick_clock.global_clock}))
        si = drain_inst.ins.sync_info
        waits = list(si.on_wait or [])
        if len(waits) > 1:
            si.on_wait[:] = waits[:1]
            for w in waits[1:]:
                d2 = nc.sync.drain()
                if d2.ins.sync_info is None:
                    d2.ins.sync_info = mybir.SyncInfo(on_wait=[w], on_update=[])
                else:
                    d2.ins.sync_info.on_wait.append(w)
        nc.all_engine_barrier()
        assert self.sems is not None
        popped = nc._tile_sem_poison_stack.pop()
        assert popped is self._sem_poison
        nc.clear_and_free_semaphores(list(self.sems.allocated().values()))
        nc.all_engine_barrier()

    tile_mod.TileContext._drain_and_barrier = _patched_drain_and_barrier

    _orig_lower_ordered = tile_mod.TileContext._lower_ordered_insts

    def _split_multiwait_lower(self, ordered):
        nc = self.nc
        for bbname, insts in ordered.items():
            newlist = []
            changed = False
            for inst in insts:
                si = getattr(inst, "sync_info", None)
                eng = getattr(inst, "engine", None)
                if (si is not None and si.on_wait and len(si.on_wait) > 1
                        and eng is not None and eng != mybir.EngineType.Unassigned
                        and inst.is_executable()):
                    waits = list(si.on_wait)
                    si.on_wait[:] = waits[-1:]
                    for w in waits[:-1]:
                        nop = mybir.InstNoOp(
                            name=nc.get_next_instruction_name(), engine=eng)
                        nop.sync_info = mybir.SyncInfo(on_wait=[w], on_update=[])
                        try:
                            nc.register_instruction(nop, overwrite=True)
                        except Exception:
                            pass
                        newlist.append(nop)
                    changed = True
                newlist.append(inst)
            if changed:
                insts[:] = newlist
        return _orig_lower_ordered(self, ordered)

    if getattr(tile_mod.TileContext._lower_ordered_insts, "__name__", "") != \
            "_split_multiwait_lower":
        tile_mod.TileContext._lower_ordered_insts = _split_multiwait_lower

    # NTFF profile hook (lets trace=True work under axon); best-effort.
    if "antenv.axon_hooks" not in sys.modules:
        try:
            from trn_agent_boot.trn_boot import _ntff_profile_via_ctypes
            hook = _ntff_profile_via_ctypes("/opt/axon/libaxon_pjrt.so")
            mod = types.ModuleType("antenv.axon_hooks")
            mod.get_axon_ntff_profile_hook = lambda: hook
            mod.set_axon_ntff_profile_hook = lambda h: None
            sys.modules["antenv.axon_hooks"] = mod
            import antenv
            antenv.axon_hooks = mod
        except Exception:
            pass


_install_env_fixes()

# ---------------------------------------------------------------------------
# Host preprocessing
# ---------------------------------------------------------------------------

def preprocess(x, edge_index, batch, n_graphs):
    N = x.shape[0]
    GPC = n_graphs // M
    src = np.concatenate([edge_index[0], np.arange(N, dtype=np.int64)])
    dst = np.concatenate([edge_index[1], np.arange(N, dtype=np.int64)])
    deg = np.bincount(dst, minlength=N).astype(np.float32)
    dis = 1.0 / np.sqrt(deg)
    norm = (dis[src] * dis[dst]).astype(np.float32)

    batch = np.asarray(batch)
    owner = (batch // GPC).astype(np.int64)
    node_start = np.searchsorted(batch, np.arange(M) * GPC)
    node_end = np.searchsorted(batch, np.arange(M) * GPC + GPC)
    n_c = node_end - node_start
    T = int(np.ceil(n_c.max() / 128))
    S = T * 128
    R = 2 * S  # rows per source region
    assert R <= 32767, R
    local_pos = np.arange(N) - node_start[owner]
    # zfull row layout matches the two half-AllGathers per layer:
    # rows [0, M*S/2) = cores' first halves, then cores' second halves.
    HS = S // 2
    padded_idx = np.where(
        local_pos < HS, owner * HS + local_pos,
        M * HS + owner * HS + (local_pos - HS)).astype(np.int64)

    eo = owner[dst]
    ld_all = local_pos[dst]
    grow = padded_idx[src]          # global row in zfull
    rg_all = grow // R              # source region
    rloc_all = grow % R             # region-local row (int16 range)

    # uniform blocks-per-(tile, region): max over cores
    cnt = np.zeros((M, T, NREG), np.int64)
    for c in range(M):
        sel = eo == c
        np.add.at(cnt[c], (ld_all[sel] // 128, rg_all[sel]), 1)
    b_u = -(-cnt.max(axis=0) // 128)          # [T, NREG] blocks
    assert (b_u.sum(axis=1) > 0).all()
    boff = np.zeros((T + 1, NREG), np.int64)  # region-block offsets by tile
    boff[1:] = np.cumsum(b_u, axis=0)
    NB_r = boff[T]                            # blocks per region
    RB = np.concatenate([[0], np.cumsum(NB_r)])  # region col offsets
    NBtot = int(RB[-1])

    G = -(-T // TC)
    # chunk block ranges per region: CB[g][r] = boff[g*TC][r]
    CB = np.stack([boff[min(g * TC, T)] for g in range(G + 1)])

    cnt_g = np.bincount(batch, minlength=n_graphs).astype(np.float32)

    in_maps = []
    for c in range(M):
        sel = np.nonzero(eo == c)[0]
        en, ld = norm[sel], ld_all[sel]
        rg, rloc = rg_all[sel], rloc_all[sel]
        tid = ld // 128
        order = np.lexsort((rg, tid))
        en, ld, rg, rloc, tid = en[order], ld[order], rg[order], rloc[order], tid[order]

        ccnt = cnt[c]                          # [T, NREG] this core's counts
        flat_cnt = ccnt.reshape(-1)
        starts = np.concatenate([[0], np.cumsum(flat_cnt)])[:-1]
        pos = np.arange(len(en)) - np.repeat(starts, flat_cnt)
        slot = (RB[rg] + boff[tid, rg]) * 128 + \
            (np.zeros(len(en), np.int64)) + pos
        # pos counts within the (tid, rg) group in sorted order

        idx_flat = np.zeros(NBtot * 128, np.int16)
        nrm_flat = np.zeros(NBtot * 128, np.float32)
        ldst_flat = np.zeros(NBtot * 128, np.float16)
        idx_flat[slot] = rloc.astype(np.int16)
        nrm_flat[slot] = en
        ldst_flat[slot] = (ld % 128).astype(np.float16)

        # idx packed for dma_gather: element i -> [i % 16, i // 16], x8 copies
        C = NBtot * 8
        pat = idx_flat.reshape(C, 16).T
        idx_pk = np.zeros((128, C), np.int16)
        for k in range(8):
            idx_pk[k * 16:(k + 1) * 16] = pat

        def to_cols(a):
            return np.ascontiguousarray(
                a.reshape(NBtot, 128).T)           # [128, NBtot]

        xT = np.zeros((128, S), np.float32)
        xs = x[node_start[c]:node_end[c]]
        xT[:, :n_c[c]] = xs.T

        gflat = np.full(T * 128, -1.0, np.float32)
        gflat[:n_c[c]] = batch[node_start[c]:node_end[c]] - c * GPC
        gcol = gflat.reshape(T, 128).T

        invcnt = np.zeros((128, 1), np.float32)
        invcnt[:GPC, 0] = 1.0 / np.maximum(cnt_g[c * GPC:(c + 1) * GPC], 1.0)

        iota = np.broadcast_to(np.arange(128, dtype=np.float32), (128, 128)).copy()
        iota16 = iota.astype(np.float16)

        in_maps.append(dict(
            xT=xT, idx=idx_pk, nrm=to_cols(nrm_flat), ldst=to_cols(ldst_flat),
            gcol=np.ascontiguousarray(gcol), invcnt=invcnt, iota=iota,
            iota16=iota16,
        ))
    meta = dict(T=T, S=S, GPC=GPC, b_u=b_u, boff=boff, RB=RB, NBtot=NBtot,
                G=G, CB=CB)
    return in_maps, meta


def make_weight_inputs(W1, b1, Wh, bh, Wout, bout):
    return dict(
        W1=np.ascontiguousarray(W1, np.float32),
        Wh=np.ascontiguousarray(Wh, np.float32),
        b14=np.ascontiguousarray(
            np.stack([b1, bh[0], bh[1], bh[2]], axis=1), np.float32),
        b5rep=np.broadcast_to(np.asarray(bh[3], np.float32), (128, H)).copy(),
        woutrep=np.broadcast_to(np.asarray(Wout, np.float32)[:, 0], (128, H)).copy(),
    ), dict(bout=np.asarray(bout, np.float32))


# ---------------------------------------------------------------------------
# Bass program
# ---------------------------------------------------------------------------

def build_nc(meta, weights):
    T, S = meta["T"], meta["S"]
    b_u, boff, RB, NBtot = meta["b_u"], meta["boff"], meta["RB"], meta["NBtot"]
    G, CB = meta["G"], meta["CB"]
    R = 2 * S
    BMAXT = int(b_u.sum(axis=1).max())            # max blocks per tile
    NBGR = int((CB[1:] - CB[:-1]).max())          # max blocks per (chunk, region)

    nc = bass.Bass("TRN2", target_bir_lowering=False, num_swdge_queues=4)

    xT_d = nc.declare_dram_parameter("xT", [128, S], F32, isOutput=False)
    idx_d = nc.declare_dram_parameter("idx", [128, NBtot * 8], I16, isOutput=False)
    nrm_d = nc.declare_dram_parameter("nrm", [128, NBtot], F32, isOutput=False)
    ldst_d = nc.declare_dram_parameter("ldst", [128, NBtot], F16, isOutput=False)
    gcol_d = nc.declare_dram_parameter("gcol", [128, T], F32, isOutput=False)
    invcnt_d = nc.declare_dram_parameter("invcnt", [128, 1], F32, isOutput=False)
    iota_d = nc.declare_dram_parameter("iota", [128, 128], F32, isOutput=False)
    iota16_d = nc.declare_dram_parameter("iota16", [128, 128], F16, isOutput=False)
    W1_d = nc.declare_dram_parameter("W1", [128, H], F32, isOutput=False)
    Wh_d = nc.declare_dram_parameter("Wh", [4, H, H], F32, isOutput=False)
    b14_d = nc.declare_dram_parameter("b14", [H, 4], F32, isOutput=False)
    b5rep_d = nc.declare_dram_parameter("b5rep", [128, H], F32, isOutput=False)
    woutrep_d = nc.declare_dram_parameter("woutrep", [128, H], F32, isOutput=False)
    out_d = nc.declare_dram_parameter("out", [128, 1], F32, isOutput=True)
    bout = float(weights["bout"][0])

    with tile.TileContext(nc) as tc:
        with contextlib.ExitStack() as ctx:
            zfull_pool = ctx.enter_context(tc.tile_pool(name="zfull", bufs=1, space="DRAM"))
            dram = ctx.enter_context(tc.tile_pool(name="dram", bufs=1, space="DRAM"))
            const = ctx.enter_context(tc.tile_pool(name="const", bufs=1))
            xp = ctx.enter_context(tc.tile_pool(name="xp", bufs=3))
            gp = ctx.enter_context(tc.tile_pool(name="gp", bufs=2))
            g16p = ctx.enter_context(tc.tile_pool(name="g16p", bufs=2))
            mp = ctx.enter_context(tc.tile_pool(name="mp", bufs=3))
            zp = ctx.enter_context(tc.tile_pool(name="zp", bufs=3))
            hp = ctx.enter_context(tc.tile_pool(name="hp", bufs=3))
            ep = ctx.enter_context(tc.tile_pool(name="ep", bufs=2))
            ps_agg = ctx.enter_context(tc.tile_pool(name="ps_agg", bufs=2, space="PSUM"))
            ps_z = ctx.enter_context(tc.tile_pool(name="ps_z", bufs=2, space="PSUM"))
            ps_misc = ctx.enter_context(tc.tile_pool(name="ps_misc", bufs=1, space="PSUM"))

            zfulls = [[zfull_pool.tile([M * (S // 2), H], F32, addr_space="Shared",
                                       name=f"zfull{k}h{h}", tag=f"zfull{k}h{h}")
                       for h in range(2)] for k in range(5)]
            bounces = [dram.tile([S, H], F32, name=f"bounce{k}", tag=f"bounce{k}")
                       for k in range(5)]

            idx_sb = const.tile([128, NBtot * 8], I16)
            nc.sync.dma_start(idx_sb[:], idx_d[:])
            nrm_sb = const.tile([128, NBtot], F32)
            nc.sync.dma_start(nrm_sb[:], nrm_d[:])
            ldst_sb = const.tile([128, NBtot], F16)
            nc.sync.dma_start(ldst_sb[:], ldst_d[:])
            gcol_sb = const.tile([128, T], F32)
            nc.sync.dma_start(gcol_sb[:], gcol_d[:])
            invcnt_sb = const.tile([128, 1], F32)
            nc.sync.dma_start(invcnt_sb[:], invcnt_d[:])
            iota_sb = const.tile([128, 128], F32)
            nc.sync.dma_start(iota_sb[:], iota_d[:])
            iota16_sb = const.tile([128, 128], F16)
            nc.sync.dma_start(iota16_sb[:], iota16_d[:])
            W1_sb = const.tile([128, H], F32)
            nc.sync.dma_start(W1_sb[:], W1_d[:])
            Wh2_sb = const.tile([H, 4 * H], F32)
            for k in range(4):
                nc.sync.dma_start(Wh2_sb[:, k * H:(k + 1) * H], Wh_d[k, :, :])
            b14_sb = const.tile([H, 4], F32)
            nc.sync.dma_start(b14_sb[:], b14_d[:])
            b5rep_sb = const.tile([128, H], F32)
            nc.sync.dma_start(b5rep_sb[:], b5rep_d[:])
            woutrep_sb = const.tile([128, H], F32)
            nc.sync.dma_start(woutrep_sb[:], woutrep_d[:])
            ident = const.tile([H, H], F32)
            make_identity(nc, ident[:])
            nc.gpsimd.load_library(library_config.mlp)
            nvals = sorted({int(CB[g + 1][r] - CB[g][r]) * 128
                            for g in range(G) for r in range(NREG)
                            if CB[g + 1][r] > CB[g][r]})
            nreg = {v: nc.gpsimd.to_reg(v) for v in nvals}

            HS = S // 2
            TH = T // 2

            def allgather_half(k, half):
                lo, hi = (0, HS) if half == 0 else (HS, S)
                nc.gpsimd.collective_compute(
                    "AllGather", mybir.AluOpType.bypass,
                    replica_groups=[list(range(M))],
                    ins=[bounces[k][lo:hi, :]],
                    outs=[zfulls[k][half][:]])

            # layer 0: z0 = x @ W1 for the local slice, then AllGather halves
            for t in range(T):
                xt = xp.tile([128, 128], F32, tag="xt")
                nc.sync.dma_start(xt[:], xT_d[:, t * 128:(t + 1) * 128])
                pz = ps_z.tile([128, H], F32, space="PSUM", tag="pz")
                nc.tensor.matmul(out=pz[:], lhsT=xt[:], rhs=W1_sb[:], start=True, stop=True)
                zt = zp.tile([128, H], F32, tag="zt")
                nc.scalar.copy(zt[:], pz[:])
                nc.sync.dma_start(bounces[0][t * 128:(t + 1) * 128, :], zt[:])
                if t == TH - 1:
                    allgather_half(0, 0)
            allgather_half(0, 1)

            # layers 1..5: bulk-gather z[src] per (chunk, region), fp16
            # messages, one-hot scatter matmul per dst tile
            for layer in range(1, 6):
                pool_acc = None
                if layer == 5:
                    pool_acc = const.tile([H, 128], F32, name="pool_acc")
                    nc.vector.memset(pool_acc[:], 0.0)
                    h5all = const.tile([128, T * H], F32, name="h5all")
                for g in range(G):
                    t0, t1 = g * TC, min((g + 1) * TC, T)
                    gbufs, g16s = [], []
                    for r in range(NREG):
                        nb = int(CB[g + 1][r] - CB[g][r])
                        if nb == 0:
                            gbufs.append(None)
                            g16s.append(None)
                            continue
                        gb = gp.tile([128, NBGR * H], F32, tag=f"g{r}")
                        c0 = int(RB[r] + CB[g][r])
                        zsrc = zfulls[layer - 1][r // 2]
                        nc.gpsimd.dma_gather(
                            out_ap=gb[:, :nb * H].rearrange("p (b e) -> p b e", e=H),
                            in_ap=zsrc[(r % 2) * R:(r % 2 + 1) * R, :],
                            idxs_ap=idx_sb[:, c0 * 8:(c0 + nb) * 8],
                            num_idxs=nb * 128,
                            num_idxs_reg=nreg[nb * 128],
                            elem_size=H,
                            single_packet=False,
                            queue_num=r,
                        )
                        g16 = g16p.tile([128, NBGR * H], F16, tag=f"h{r}")
                        nrm3 = nrm_sb[:, c0:c0 + nb, None].to_broadcast([128, nb, H])
                        nc.vector.tensor_tensor(
                            out=g16[:, :nb * H].rearrange("p (b e) -> p b e", e=H),
                            in0=gb[:, :nb * H].rearrange("p (b e) -> p b e", e=H),
                            in1=nrm3, op=mybir.AluOpType.mult)
                        gbufs.append(gb)
                        g16s.append(g16)

                    for t in range(t0, t1):
                        sumb = int(b_u[t].sum())
                        m01 = mp.tile([128, BMAXT * 128], F16, tag="m01")
                        jb = 0
                        for r in range(NREG):
                            b = int(b_u[t][r])
                            if b == 0:
                                continue
                            cc = int(RB[r] + boff[t][r])
                            ldst3 = ldst_sb[:, cc:cc + b, None].to_broadcast([128, b, 128])
                            iota3 = iota16_sb[:, None, :].to_broadcast([128, b, 128])
                            m3 = m01[:, jb * 128:(jb + b) * 128].rearrange(
                                "p (b q) -> p b q", b=b)
                            nc.vector.tensor_tensor(out=m3, in0=iota3, in1=ldst3,
                                                    op=mybir.AluOpType.is_equal)
                            jb += b
                        if layer < 5:
                            pT = ps_agg.tile([H, 128], F32, space="PSUM", tag="pT")
                            jb = 0
                            for r in range(NREG):
                                b = int(b_u[t][r])
                                if b == 0:
                                    continue
                                lo = int(boff[t][r] - CB[g][r])
                                for j in range(b):
                                    nc.tensor.matmul(
                                        out=pT[:],
                                        lhsT=g16s[r][:, (lo + j) * H:(lo + j + 1) * H],
                                        rhs=m01[:, jb * 128:(jb + 1) * 128],
                                        start=(jb == 0), stop=(jb == sumb - 1))
                                    jb += 1
                            hT = hp.tile([H, 128], F32, tag="hT")
                            nc.scalar.activation(hT[:], pT[:],
                                                 mybir.ActivationFunctionType.Relu,
                                                 bias=b14_sb[:, layer - 1:layer])
                            pz = ps_z.tile([128, H], F32, space="PSUM", tag="pz")
                            nc.tensor.matmul(out=pz[:], lhsT=hT[:],
                                             rhs=Wh2_sb[:, (layer - 1) * H:layer * H],
                                             start=True, stop=True)
                            zt = zp.tile([128, H], F32, tag="zt2")
                            nc.scalar.copy(zt[:], pz[:])
                            nc.sync.dma_start(bounces[layer][t * 128:(t + 1) * 128, :], zt[:])
                        else:
                            p5 = ps_agg.tile([128, H], F32, space="PSUM", tag="pT")
                            jb = 0
                            for r in range(NREG):
                                b = int(b_u[t][r])
                                if b == 0:
                                    continue
                                lo = int(boff[t][r] - CB[g][r])
                                for j in range(b):
                                    nc.tensor.matmul(
                                        out=p5[:],
                                        lhsT=m01[:, jb * 128:(jb + 1) * 128],
                                        rhs=g16s[r][:, (lo + j) * H:(lo + j + 1) * H],
                                        start=(jb == 0), stop=(jb == sumb - 1))
                                    jb += 1
                            h5 = h5all[:, t * H:(t + 1) * H]
                            nc.vector.tensor_tensor(out=h5, in0=p5[:], in1=b5rep_sb[:],
                                                    op=mybir.AluOpType.add)
                            nc.scalar.activation(h5, h5, mybir.ActivationFunctionType.Relu)
                    if layer < 5 and t1 >= TH and t0 < TH:
                        allgather_half(layer, 0)
                if layer < 5:
                    allgather_half(layer, 1)

            # mean-pool per graph (one-hot matmul per tile, accumulate in SBUF)
            for t in range(T):
                pt = mp.tile([128, 128], F32, tag="pt")
                nc.vector.tensor_tensor(
                    out=pt[:], in0=iota_sb[:],
                    in1=gcol_sb[:, t:t + 1].to_broadcast([128, 128]),
                    op=mybir.AluOpType.is_equal)
                ppool = ps_misc.tile([H, 128], F32, space="PSUM", tag="pool")
                nc.tensor.matmul(out=ppool[:], lhsT=h5all[:, t * H:(t + 1) * H],
                                 rhs=pt[:], start=True, stop=True)
                nc.vector.tensor_tensor(out=pool_acc[:], in0=pool_acc[:],
                                        in1=ppool[:], op=mybir.AluOpType.add)

            # transpose, scale by 1/cnt, layernorm, output head
            ptr = ps_z.tile([128, H], F32, space="PSUM", tag="pz")
            nc.tensor.transpose(out=ptr[:], in_=pool_acc[:], identity=ident[:])
            pooled = ep.tile([128, H], F32, tag="pooled")
            nc.vector.tensor_scalar(out=pooled[:], in0=ptr[:], scalar1=invcnt_sb[:, 0:1],
                                    scalar2=None, op0=mybir.AluOpType.mult)
            mu = ep.tile([128, 1], F32, tag="mu")
            nc.vector.tensor_reduce(out=mu[:], in_=pooled[:], axis=mybir.AxisListType.X,
                                    op=mybir.AluOpType.add)
            nc.vector.tensor_scalar(out=mu[:], in0=mu[:], scalar1=1.0 / H, scalar2=None,
                                    op0=mybir.AluOpType.mult)
            xc = ep.tile([128, H], F32, tag="xc")
            nc.vector.tensor_scalar(out=xc[:], in0=pooled[:], scalar1=mu[:, 0:1],
                                    scalar2=None, op0=mybir.AluOpType.subtract)
            sq = ep.tile([128, H], F32, tag="sq")
            nc.scalar.activation(sq[:], xc[:], mybir.ActivationFunctionType.Square)
            var = ep.tile([128, 1], F32, tag="var")
            nc.vector.tensor_reduce(out=var[:], in_=sq[:], axis=mybir.AxisListType.X,
                                    op=mybir.AluOpType.add)
            nc.vector.tensor_scalar(out=var[:], in0=var[:], scalar1=1.0 / H, scalar2=None,
                                    op0=mybir.AluOpType.mult)
            eps_col = ep.tile([128, 1], F32, tag="eps")
            nc.gpsimd.memset(eps_col[:], 1e-5)
            std = ep.tile([128, 1], F32, tag="std")
            nc.scalar.activation(std[:], var[:], mybir.ActivationFunctionType.Sqrt,
                                 bias=eps_col[:, 0:1])
            rstd = ep.tile([128, 1], F32, tag="rstd")
            nc.vector.reciprocal(rstd[:], std[:])
            ln = ep.tile([128, H], F32, tag="ln")
            nc.vector.tensor_scalar(out=ln[:], in0=xc[:], scalar1=rstd[:, 0:1],
                                    scalar2=None, op0=mybir.AluOpType.mult)
            y = ep.tile([128, H], F32, tag="y")
            nc.vector.tensor_tensor(out=y[:], in0=ln[:], in1=woutrep_sb[:],
                                    op=mybir.AluOpType.mult)
            yr = ep.tile([128, 1], F32, tag="yr")
            nc.vector.tensor_reduce(out=yr[:], in_=y[:], axis=mybir.AxisListType.X,
                                    op=mybir.AluOpType.add)
            nc.vector.tensor_scalar(out=yr[:], in0=yr[:], scalar1=bout, scalar2=None,
                                    op0=mybir.AluOpType.add)
            nc.sync.dma_start(out_d[:], yr[:])
    mybir.codegen_inst_isa_subclasses(nc)
    return nc


# ---------------------------------------------------------------------------
# Entry point
# ---------------------------------------------------------------------------

def kernel(x, edge_index, batch, W1, b1, Wh, bh, Wout, bout):
    from concourse.bass_utils import run_bass_kernel_spmd

    x = np.asarray(x, np.float32)
    edge_index = np.asarray(edge_index)
    batch = np.asarray(batch)
    n_graphs = 1000

    in_maps, meta = preprocess(x, edge_index, batch, n_graphs)
    wmaps, wmeta = make_weight_inputs(W1, b1, Wh, bh, Wout, bout)
    nc = build_nc(meta, dict(bout=wmeta["bout"]))
    for im in in_maps:
        im.update(wmaps)

    import time
    last_err = None
    for attempt in range(3):
        try:
            res = run_bass_kernel_spmd(nc, in_maps, core_ids=list(range(M)))
            break
        except Exception as e:  # transient terminal hiccups / device recovery
            last_err = e
            time.sleep(30 * (attempt + 1))
    else:
        raise last_err

    GPC = meta["GPC"]
    out = np.concatenate([res.results[c]["out"][:GPC] for c in range(M)], axis=0)
    return np.ascontiguousarray(out, np.float32)


# revision 17
# speedup vs baseline: 1.2550x; 1.0944x over previous
"""GCN-5 message-passing kernel for Trainium2, 8-core SPMD Bass/Tile.

Strategy (graph-parallel, per the sharding hint):
  - batch is sorted, so graphs are contiguous node ranges.  Core c owns graphs
    [125c, 125(c+1)) and their nodes; edges are assigned to the core owning the
    dst node.  Pooling / layernorm / output head are fully local per core.
  - Per layer each core computes z = h @ W for its own node slice on the
    TensorEngine, the slices are AllGathered (device collective), and each core
    gathers z[src] rows for its edges with bulk dma_gather (SWDGE): one call
    per (source 2-core region, chunk of TC dst tiles), thousands of rows per
    instruction instead of one 128-row indirect DMA per edge block.  Source
    regions span 2 cores (2S = 25600 rows) so row indices fit dma_gather's
    int16 index format.
  - Scatter-add per 128-node dst tile is a one-hot matmul on the TensorEngine
    with PSUM accumulation.  The gathered messages and one-hot masks are fp16
    (2x PE/DVE rate; one-hot values and dst positions are exact in fp16).
    Edges are pre-sorted by (dst tile, src region) and padded to 128-slot
    blocks on the host; padded slots gather region row 0 with norm=0.
  - Block structure (blocks per tile/region) is the max over cores so the
    single SPMD program fits all 8 cores; per-core padding lives in the data.
  - The symmetric normalization deg^-1/2 and all index inputs are precomputed
    on the host from edge_index/batch.
"""
import sys
import types
import contextlib

import numpy as np

sys.path.insert(0, "/opt/trn_rl_repo")

import concourse.bass as bass
import concourse.tile as tile
from concourse import mybir, library_config
from concourse.masks import make_identity
from concourse.vector_clock import ScopedClock

F32 = mybir.dt.float32
F16 = mybir.dt.float16
I16 = mybir.dt.int16
M = 8  # NeuronCores
H = 64
NREG = 4   # source regions of 2 cores each (2S rows <= int16 max)
TC = 6     # dst tiles per gather chunk

# ---------------------------------------------------------------------------
# Environment fixes for this container
# ---------------------------------------------------------------------------

def _install_env_fixes():
    import concourse.tile as tile_mod

    def _patched_drain_and_barrier(self, tick_clock, wait_clock):
        # this walrus build allows a single sync-wait per TPB_CTRL Drain;
        # split the Tile tail-drain's waits across multiple drains.
        nc = self.nc
        drain_inst = nc.sync.drain()
        wait_clock.add_sem_waits(drain_inst.ins,
                                 ScopedClock({None: tick_clock.global_clock}))
        si = drain_inst.ins.sync_info
        waits = list(si.on_wait or [])
        if len(waits) > 1:
            si.on_wait[:] = waits[:1]
            for w in waits[1:]:
                d2 = nc.sync.drain()
                if d2.ins.sync_info is None:
                    d2.ins.sync_info = mybir.SyncInfo(on_wait=[w], on_update=[])
                else:
                    d2.ins.sync_info.on_wait.append(w)
        nc.all_engine_barrier()
        assert self.sems is not None
        popped = nc._tile_sem_poison_stack.pop()
        assert popped is self._sem_poison
        nc.clear_and_free_semaphores(list(self.sems.allocated().values()))
        nc.all_engine_barrier()

    tile_mod.TileContext._drain_and_barrier = _patched_drain_and_barrier

    _orig_lower_ordered = tile_mod.TileContext._lower_ordered_insts

    def _split_multiwait_lower(self, ordered):
        nc = self.nc
        for bbname, insts in ordered.items():
            newlist = []
            changed = False
            for inst in insts:
                si = getattr(inst, "sync_info", None)
                eng = getattr(inst, "engine", None)
                if (si is not None and si.on_wait and len(si.on_wait) > 1
                        and eng is not None and eng != mybir.EngineType.Unassigned
                        and inst.is_executable()):
                    waits = list(si.on_wait)
                    si.on_wait[:] = waits[-1:]
                    for w in waits[:-1]:
                        nop = mybir.InstNoOp(
                            name=nc.get_next_instruction_name(), engine=eng)
                        nop.sync_info = mybir.SyncInfo(on_wait=[w], on_update=[])
                        try:
                            nc.register_instruction(nop, overwrite=True)
                        except Exception:
                            pass
                        newlist.append(nop)
                    changed = True
                newlist.append(inst)
            if changed:
                insts[:] = newlist
        return _orig_lower_ordered(self, ordered)

    if getattr(tile_mod.TileContext._lower_ordered_insts, "__name__", "") != \
            "_split_multiwait_lower":
        tile_mod.TileContext._lower_ordered_insts = _split_multiwait_lower

    # NTFF profile hook (lets trace=True work under axon); best-effort.
    if "antenv.axon_hooks" not in sys.modules:
        try:
            from trn_agent_boot.trn_boot import _ntff_profile_via_ctypes
            hook = _ntff_profile_via_ctypes("/opt/axon/libaxon_pjrt.so")
            mod = types.ModuleType("antenv.axon_hooks")
            mod.get_axon_ntff_profile_hook = lambda: hook
            mod.set_axon_ntff_profile_hook = lambda h: None
            sys.modules["antenv.axon_hooks"] = mod
            import antenv
            antenv.axon_hooks = mod
        except Exception:
            pass


_install_env_fixes()

# ---------------------------------------------------------------------------
# Host preprocessing
# ---------------------------------------------------------------------------

def preprocess(x, edge_index, batch, n_graphs):
    N = x.shape[0]
    GPC = n_graphs // M
    src = np.concatenate([edge_index[0], np.arange(N, dtype=np.int64)])
    dst = np.concatenate([edge_index[1], np.arange(N, dtype=np.int64)])
    deg = np.bincount(dst, minlength=N).astype(np.float32)
    dis = 1.0 / np.sqrt(deg)
    norm = (dis[src] * dis[dst]).astype(np.float32)

    batch = np.asarray(batch)
    owner = (batch // GPC).astype(np.int64)
    node_start = np.searchsorted(batch, np.arange(M) * GPC)
    node_end = np.searchsorted(batch, np.arange(M) * GPC + GPC)
    n_c = node_end - node_start
    T = int(np.ceil(n_c.max() / 128))
    S = T * 128
    R = 2 * S  # rows per source region
    assert R <= 32767, R
    local_pos = np.arange(N) - node_start[owner]
    # zfull row layout matches the two half-AllGathers per layer:
    # rows [0, M*S/2) = cores' first halves, then cores' second halves.
    HS = S // 2
    padded_idx = np.where(
        local_pos < HS, owner * HS + local_pos,
        M * HS + owner * HS + (local_pos - HS)).astype(np.int64)

    eo = owner[dst]
    ld_all = local_pos[dst]
    grow = padded_idx[src]          # global row in zfull
    rg_all = grow // R              # source region
    rloc_all = grow % R             # region-local row (int16 range)

    # uniform blocks-per-(tile, region): max over cores
    cnt = np.zeros((M, T, NREG), np.int64)
    for c in range(M):
        sel = eo == c
        np.add.at(cnt[c], (ld_all[sel] // 128, rg_all[sel]), 1)
    b_u = -(-cnt.max(axis=0) // 128)          # [T, NREG] blocks
    assert (b_u.sum(axis=1) > 0).all()
    boff = np.zeros((T + 1, NREG), np.int64)  # region-block offsets by tile
    boff[1:] = np.cumsum(b_u, axis=0)
    NB_r = boff[T]                            # blocks per region
    RB = np.concatenate([[0], np.cumsum(NB_r)])  # region col offsets
    NBtot = int(RB[-1])

    G = -(-T // TC)
    # chunk block ranges per region: CB[g][r] = boff[g*TC][r]
    CB = np.stack([boff[min(g * TC, T)] for g in range(G + 1)])

    cnt_g = np.bincount(batch, minlength=n_graphs).astype(np.float32)

    in_maps = []
    for c in range(M):
        sel = np.nonzero(eo == c)[0]
        en, ld = norm[sel], ld_all[sel]
        rg, rloc = rg_all[sel], rloc_all[sel]
        tid = ld // 128
        order = np.lexsort((rg, tid))
        en, ld, rg, rloc, tid = en[order], ld[order], rg[order], rloc[order], tid[order]

        ccnt = cnt[c]                          # [T, NREG] this core's counts
        flat_cnt = ccnt.reshape(-1)
        starts = np.concatenate([[0], np.cumsum(flat_cnt)])[:-1]
        pos = np.arange(len(en)) - np.repeat(starts, flat_cnt)
        slot = (RB[rg] + boff[tid, rg]) * 128 + \
            (np.zeros(len(en), np.int64)) + pos
        # pos counts within the (tid, rg) group in sorted order

        idx_flat = np.zeros(NBtot * 128, np.int16)
        nrm_flat = np.zeros(NBtot * 128, np.float32)
        ldst_flat = np.zeros(NBtot * 128, np.float16)
        idx_flat[slot] = rloc.astype(np.int16)
        nrm_flat[slot] = en
        ldst_flat[slot] = (ld % 128).astype(np.float16)

        # idx packed for dma_gather: element i -> [i % 16, i // 16], x8 copies
        C = NBtot * 8
        pat = idx_flat.reshape(C, 16).T
        idx_pk = np.zeros((128, C), np.int16)
        for k in range(8):
            idx_pk[k * 16:(k + 1) * 16] = pat

        def to_cols(a):
            return np.ascontiguousarray(
                a.reshape(NBtot, 128).T)           # [128, NBtot]

        xT = np.zeros((128, S), np.float32)
        xs = x[node_start[c]:node_end[c]]
        xT[:, :n_c[c]] = xs.T

        gflat = np.full(T * 128, -1.0, np.float32)
        gflat[:n_c[c]] = batch[node_start[c]:node_end[c]] - c * GPC
        gcol = gflat.reshape(T, 128).T

        invcnt = np.zeros((128, 1), np.float32)
        invcnt[:GPC, 0] = 1.0 / np.maximum(cnt_g[c * GPC:(c + 1) * GPC], 1.0)

        iota = np.broadcast_to(np.arange(128, dtype=np.float32), (128, 128)).copy()
        iota16 = iota.astype(np.float16)

        in_maps.append(dict(
            xT=xT, idx=idx_pk, nrm=to_cols(nrm_flat), ldst=to_cols(ldst_flat),
            gcol=np.ascontiguousarray(gcol), invcnt=invcnt, iota=iota,
            iota16=iota16,
        ))
    meta = dict(T=T, S=S, GPC=GPC, b_u=b_u, boff=boff, RB=RB, NBtot=NBtot,
                G=G, CB=CB)
    return in_maps, meta


def make_weight_inputs(W1, b1, Wh, bh, Wout, bout):
    return dict(
        W1=np.ascontiguousarray(W1, np.float32),
        Wh=np.ascontiguousarray(Wh, np.float32),
        b14=np.ascontiguousarray(
            np.stack([b1, bh[0], bh[1], bh[2]], axis=1), np.float32),
        b5rep=np.broadcast_to(np.asarray(bh[3], np.float32), (128, H)).copy(),
        woutrep=np.broadcast_to(np.asarray(Wout, np.float32)[:, 0], (128, H)).copy(),
    ), dict(bout=np.asarray(bout, np.float32))


# ---------------------------------------------------------------------------
# Bass program
# ---------------------------------------------------------------------------

def build_nc(meta, weights):
    T, S = meta["T"], meta["S"]
    b_u, boff, RB, NBtot = meta["b_u"], meta["boff"], meta["RB"], meta["NBtot"]
    G, CB = meta["G"], meta["CB"]
    R = 2 * S
    BMAXT = int(b_u.sum(axis=1).max())            # max blocks per tile
    NBGR = int((CB[1:] - CB[:-1]).max())          # max blocks per (chunk, region)

    nc = bass.Bass("TRN2", target_bir_lowering=False, num_swdge_queues=4)

    xT_d = nc.declare_dram_parameter("xT", [128, S], F32, isOutput=False)
    idx_d = nc.declare_dram_parameter("idx", [128, NBtot * 8], I16, isOutput=False)
    nrm_d = nc.declare_dram_parameter("nrm", [128, NBtot], F32, isOutput=False)
    ldst_d = nc.declare_dram_parameter("ldst", [128, NBtot], F16, isOutput=False)
    gcol_d = nc.declare_dram_parameter("gcol", [128, T], F32, isOutput=False)
    invcnt_d = nc.declare_dram_parameter("invcnt", [128, 1], F32, isOutput=False)
    iota_d = nc.declare_dram_parameter("iota", [128, 128], F32, isOutput=False)
    iota16_d = nc.declare_dram_parameter("iota16", [128, 128], F16, isOutput=False)
    W1_d = nc.declare_dram_parameter("W1", [128, H], F32, isOutput=False)
    Wh_d = nc.declare_dram_parameter("Wh", [4, H, H], F32, isOutput=False)
    b14_d = nc.declare_dram_parameter("b14", [H, 4], F32, isOutput=False)
    b5rep_d = nc.declare_dram_parameter("b5rep", [128, H], F32, isOutput=False)
    woutrep_d = nc.declare_dram_parameter("woutrep", [128, H], F32, isOutput=False)
    out_d = nc.declare_dram_parameter("out", [128, 1], F32, isOutput=True)
    bout = float(weights["bout"][0])

    with tile.TileContext(nc) as tc:
        with contextlib.ExitStack() as ctx:
            zfull_pool = ctx.enter_context(tc.tile_pool(name="zfull", bufs=1, space="DRAM"))
            dram = ctx.enter_context(tc.tile_pool(name="dram", bufs=1, space="DRAM"))
            const = ctx.enter_context(tc.tile_pool(name="const", bufs=1))
            xp = ctx.enter_context(tc.tile_pool(name="xp", bufs=3))
            gp = ctx.enter_context(tc.tile_pool(name="gp", bufs=2))
            g16p = ctx.enter_context(tc.tile_pool(name="g16p", bufs=2))
            mp = ctx.enter_context(tc.tile_pool(name="mp", bufs=TC + 1))
            zp = ctx.enter_context(tc.tile_pool(name="zp", bufs=3))
            hp = ctx.enter_context(tc.tile_pool(name="hp", bufs=3))
            ep = ctx.enter_context(tc.tile_pool(name="ep", bufs=2))
            ps_agg = ctx.enter_context(tc.tile_pool(name="ps_agg", bufs=2, space="PSUM"))
            ps_z = ctx.enter_context(tc.tile_pool(name="ps_z", bufs=2, space="PSUM"))
            ps_misc = ctx.enter_context(tc.tile_pool(name="ps_misc", bufs=1, space="PSUM"))

            zfulls = [[zfull_pool.tile([M * (S // 2), H], F32, addr_space="Shared",
                                       name=f"zfull{k}h{h}", tag=f"zfull{k}h{h}")
                       for h in range(2)] for k in range(5)]
            bounces = [dram.tile([S, H], F32, name=f"bounce{k}", tag=f"bounce{k}")
                       for k in range(5)]

            idx_sb = const.tile([128, NBtot * 8], I16)
            nc.sync.dma_start(idx_sb[:], idx_d[:])
            nrm_sb = const.tile([128, NBtot], F32)
            nc.sync.dma_start(nrm_sb[:], nrm_d[:])
            ldst_sb = const.tile([128, NBtot], F16)
            nc.sync.dma_start(ldst_sb[:], ldst_d[:])
            gcol_sb = const.tile([128, T], F32)
            nc.sync.dma_start(gcol_sb[:], gcol_d[:])
            invcnt_sb = const.tile([128, 1], F32)
            nc.sync.dma_start(invcnt_sb[:], invcnt_d[:])
            iota_sb = const.tile([128, 128], F32)
            nc.sync.dma_start(iota_sb[:], iota_d[:])
            iota16_sb = const.tile([128, 128], F16)
            nc.sync.dma_start(iota16_sb[:], iota16_d[:])
            W1_sb = const.tile([128, H], F32)
            nc.sync.dma_start(W1_sb[:], W1_d[:])
            Wh2_sb = const.tile([H, 4 * H], F32)
            for k in range(4):
                nc.sync.dma_start(Wh2_sb[:, k * H:(k + 1) * H], Wh_d[k, :, :])
            b14_sb = const.tile([H, 4], F32)
            nc.sync.dma_start(b14_sb[:], b14_d[:])
            b5rep_sb = const.tile([128, H], F32)
            nc.sync.dma_start(b5rep_sb[:], b5rep_d[:])
            woutrep_sb = const.tile([128, H], F32)
            nc.sync.dma_start(woutrep_sb[:], woutrep_d[:])
            ident = const.tile([H, H], F32)
            make_identity(nc, ident[:])
            nc.gpsimd.load_library(library_config.mlp)
            nvals = sorted({int(CB[g + 1][r] - CB[g][r]) * 128
                            for g in range(G) for r in range(NREG)
                            if CB[g + 1][r] > CB[g][r]})
            nreg = {v: nc.gpsimd.to_reg(v) for v in nvals}

            HS = S // 2
            TH = T // 2

            def allgather_half(k, half):
                lo, hi = (0, HS) if half == 0 else (HS, S)
                nc.gpsimd.collective_compute(
                    "AllGather", mybir.AluOpType.bypass,
                    replica_groups=[list(range(M))],
                    ins=[bounces[k][lo:hi, :]],
                    outs=[zfulls[k][half][:]])

            # layer 0: z0 = x @ W1 for the local slice, then AllGather halves
            for t in range(T):
                xt = xp.tile([128, 128], F32, tag="xt")
                nc.sync.dma_start(xt[:], xT_d[:, t * 128:(t + 1) * 128])
                pz = ps_z.tile([128, H], F32, space="PSUM", tag="pz")
                nc.tensor.matmul(out=pz[:], lhsT=xt[:], rhs=W1_sb[:], start=True, stop=True)
                zt = zp.tile([128, H], F32, tag="zt")
                nc.scalar.copy(zt[:], pz[:])
                nc.sync.dma_start(bounces[0][t * 128:(t + 1) * 128, :], zt[:])
                if t == TH - 1:
                    allgather_half(0, 0)
            allgather_half(0, 1)

            # layers 1..5: bulk-gather z[src] per (chunk, region), fp16
            # messages, one-hot scatter matmul per dst tile
            for layer in range(1, 6):
                pool_acc = None
                if layer == 5:
                    pool_acc = const.tile([H, 128], F32, name="pool_acc")
                    nc.vector.memset(pool_acc[:], 0.0)
                    h5all = const.tile([128, T * H], F32, name="h5all")
                for g in range(G):
                    t0, t1 = g * TC, min((g + 1) * TC, T)
                    gbufs = []
                    for r in range(NREG):
                        nb = int(CB[g + 1][r] - CB[g][r])
                        if nb == 0:
                            gbufs.append(None)
                            continue
                        gb = gp.tile([128, NBGR * H], F32, tag=f"g{r}")
                        c0 = int(RB[r] + CB[g][r])
                        zsrc = zfulls[layer - 1][r // 2]
                        nc.gpsimd.dma_gather(
                            out_ap=gb[:, :nb * H].rearrange("p (b e) -> p b e", e=H),
                            in_ap=zsrc[(r % 2) * R:(r % 2 + 1) * R, :],
                            idxs_ap=idx_sb[:, c0 * 8:(c0 + nb) * 8],
                            num_idxs=nb * 128,
                            num_idxs_reg=nreg[nb * 128],
                            elem_size=H,
                            single_packet=False,
                            queue_num=r,
                        )
                        gbufs.append(gb)

                    # one-hot masks need no gathered data: build them while
                    # the gathers land so the vector queue never stalls
                    m01s = {}
                    for t in range(t0, t1):
                        m01 = mp.tile([128, BMAXT * 128], F16, tag="m01")
                        jb = 0
                        for r in range(NREG):
                            b = int(b_u[t][r])
                            if b == 0:
                                continue
                            cc = int(RB[r] + boff[t][r])
                            ldst3 = ldst_sb[:, cc:cc + b, None].to_broadcast([128, b, 128])
                            iota3 = iota16_sb[:, None, :].to_broadcast([128, b, 128])
                            m3 = m01[:, jb * 128:(jb + b) * 128].rearrange(
                                "p (b q) -> p b q", b=b)
                            nc.vector.tensor_tensor(out=m3, in0=iota3, in1=ldst3,
                                                    op=mybir.AluOpType.is_equal)
                            jb += b
                        m01s[t] = m01

                    g16s = []
                    for r in range(NREG):
                        nb = int(CB[g + 1][r] - CB[g][r])
                        if nb == 0:
                            g16s.append(None)
                            continue
                        c0 = int(RB[r] + CB[g][r])
                        g16 = g16p.tile([128, NBGR * H], F16, tag=f"h{r}")
                        nrm3 = nrm_sb[:, c0:c0 + nb, None].to_broadcast([128, nb, H])
                        nc.vector.tensor_tensor(
                            out=g16[:, :nb * H].rearrange("p (b e) -> p b e", e=H),
                            in0=gbufs[r][:, :nb * H].rearrange("p (b e) -> p b e", e=H),
                            in1=nrm3, op=mybir.AluOpType.mult)
                        g16s.append(g16)

                    for t in range(t0, t1):
                        sumb = int(b_u[t].sum())
                        m01 = m01s[t]
                        if layer < 5:
                            pT = ps_agg.tile([H, 128], F32, space="PSUM", tag="pT")
                            jb = 0
                            for r in range(NREG):
                                b = int(b_u[t][r])
                                if b == 0:
                                    continue
                                lo = int(boff[t][r] - CB[g][r])
                                for j in range(b):
                                    nc.tensor.matmul(
                                        out=pT[:],
                                        lhsT=g16s[r][:, (lo + j) * H:(lo + j + 1) * H],
                                        rhs=m01[:, jb * 128:(jb + 1) * 128],
                                        start=(jb == 0), stop=(jb == sumb - 1))
                                    jb += 1
                            hT = hp.tile([H, 128], F32, tag="hT")
                            nc.scalar.activation(hT[:], pT[:],
                                                 mybir.ActivationFunctionType.Relu,
                                                 bias=b14_sb[:, layer - 1:layer])
                            pz = ps_z.tile([128, H], F32, space="PSUM", tag="pz")
                            nc.tensor.matmul(out=pz[:], lhsT=hT[:],
                                             rhs=Wh2_sb[:, (layer - 1) * H:layer * H],
                                             start=True, stop=True)
                            zt = zp.tile([128, H], F32, tag="zt2")
                            nc.scalar.copy(zt[:], pz[:])
                            nc.sync.dma_start(bounces[layer][t * 128:(t + 1) * 128, :], zt[:])
                        else:
                            p5 = ps_agg.tile([128, H], F32, space="PSUM", tag="pT")
                            jb = 0
                            for r in range(NREG):
                                b = int(b_u[t][r])
                                if b == 0:
                                    continue
                                lo = int(boff[t][r] - CB[g][r])
                                for j in range(b):
                                    nc.tensor.matmul(
                                        out=p5[:],
                                        lhsT=m01[:, jb * 128:(jb + 1) * 128],
                                        rhs=g16s[r][:, (lo + j) * H:(lo + j + 1) * H],
                                        start=(jb == 0), stop=(jb == sumb - 1))
                                    jb += 1
                            h5 = h5all[:, t * H:(t + 1) * H]
                            nc.vector.tensor_tensor(out=h5, in0=p5[:], in1=b5rep_sb[:],
                                                    op=mybir.AluOpType.add)
                            nc.scalar.activation(h5, h5, mybir.ActivationFunctionType.Relu)
                    if layer < 5 and t1 >= TH and t0 < TH:
                        allgather_half(layer, 0)
                if layer < 5:
                    allgather_half(layer, 1)

            # mean-pool per graph (one-hot matmul per tile, accumulate in SBUF)
            for t in range(T):
                pt = mp.tile([128, 128], F32, tag="pt")
                nc.vector.tensor_tensor(
                    out=pt[:], in0=iota_sb[:],
                    in1=gcol_sb[:, t:t + 1].to_broadcast([128, 128]),
                    op=mybir.AluOpType.is_equal)
                ppool = ps_misc.tile([H, 128], F32, space="PSUM", tag="pool")
                nc.tensor.matmul(out=ppool[:], lhsT=h5all[:, t * H:(t + 1) * H],
                                 rhs=pt[:], start=True, stop=True)
                nc.vector.tensor_tensor(out=pool_acc[:], in0=pool_acc[:],
                                        in1=ppool[:], op=mybir.AluOpType.add)

            # transpose, scale by 1/cnt, layernorm, output head
            ptr = ps_z.tile([128, H], F32, space="PSUM", tag="pz")
            nc.tensor.transpose(out=ptr[:], in_=pool_acc[:], identity=ident[:])
            pooled = ep.tile([128, H], F32, tag="pooled")
            nc.vector.tensor_scalar(out=pooled[:], in0=ptr[:], scalar1=invcnt_sb[:, 0:1],
                                    scalar2=None, op0=mybir.AluOpType.mult)
            mu = ep.tile([128, 1], F32, tag="mu")
            nc.vector.tensor_reduce(out=mu[:], in_=pooled[:], axis=mybir.AxisListType.X,
                                    op=mybir.AluOpType.add)
            nc.vector.tensor_scalar(out=mu[:], in0=mu[:], scalar1=1.0 / H, scalar2=None,
                                    op0=mybir.AluOpType.mult)
            xc = ep.tile([128, H], F32, tag="xc")
            nc.vector.tensor_scalar(out=xc[:], in0=pooled[:], scalar1=mu[:, 0:1],
                                    scalar2=None, op0=mybir.AluOpType.subtract)
            sq = ep.tile([128, H], F32, tag="sq")
            nc.scalar.activation(sq[:], xc[:], mybir.ActivationFunctionType.Square)
            var = ep.tile([128, 1], F32, tag="var")
            nc.vector.tensor_reduce(out=var[:], in_=sq[:], axis=mybir.AxisListType.X,
                                    op=mybir.AluOpType.add)
            nc.vector.tensor_scalar(out=var[:], in0=var[:], scalar1=1.0 / H, scalar2=None,
                                    op0=mybir.AluOpType.mult)
            eps_col = ep.tile([128, 1], F32, tag="eps")
            nc.gpsimd.memset(eps_col[:], 1e-5)
            std = ep.tile([128, 1], F32, tag="std")
            nc.scalar.activation(std[:], var[:], mybir.ActivationFunctionType.Sqrt,
                                 bias=eps_col[:, 0:1])
            rstd = ep.tile([128, 1], F32, tag="rstd")
            nc.vector.reciprocal(rstd[:], std[:])
            ln = ep.tile([128, H], F32, tag="ln")
            nc.vector.tensor_scalar(out=ln[:], in0=xc[:], scalar1=rstd[:, 0:1],
                                    scalar2=None, op0=mybir.AluOpType.mult)
            y = ep.tile([128, H], F32, tag="y")
            nc.vector.tensor_tensor(out=y[:], in0=ln[:], in1=woutrep_sb[:],
                                    op=mybir.AluOpType.mult)
            yr = ep.tile([128, 1], F32, tag="yr")
            nc.vector.tensor_reduce(out=yr[:], in_=y[:], axis=mybir.AxisListType.X,
                                    op=mybir.AluOpType.add)
            nc.vector.tensor_scalar(out=yr[:], in0=yr[:], scalar1=bout, scalar2=None,
                                    op0=mybir.AluOpType.add)
            nc.sync.dma_start(out_d[:], yr[:])
    mybir.codegen_inst_isa_subclasses(nc)
    return nc


# ---------------------------------------------------------------------------
# Entry point
# ---------------------------------------------------------------------------

def kernel(x, edge_index, batch, W1, b1, Wh, bh, Wout, bout):
    from concourse.bass_utils import run_bass_kernel_spmd

    x = np.asarray(x, np.float32)
    edge_index = np.asarray(edge_index)
    batch = np.asarray(batch)
    n_graphs = 1000

    in_maps, meta = preprocess(x, edge_index, batch, n_graphs)
    wmaps, wmeta = make_weight_inputs(W1, b1, Wh, bh, Wout, bout)
    nc = build_nc(meta, dict(bout=wmeta["bout"]))
    for im in in_maps:
        im.update(wmaps)

    import time
    last_err = None
    for attempt in range(3):
        try:
            res = run_bass_kernel_spmd(nc, in_maps, core_ids=list(range(M)))
            break
        except Exception as e:  # transient terminal hiccups / device recovery
            last_err = e
            time.sleep(30 * (attempt + 1))
    else:
        raise last_err

    GPC = meta["GPC"]
    out = np.concatenate([res.results[c]["out"][:GPC] for c in range(M)], axis=0)
    return np.ascontiguousarray(out, np.float32)
